# revision 12
# baseline (speedup 1.0000x reference)
"""Trainium2 Bass kernel for nn_MemEffAttn (T=1024, B=4, E=1024, H=16, D=64).

Sharding (8 cores): head-parallel attention (2 heads x 4 batches per core),
Megatron-style column-sharded Wq/Wk/Wv, row-sharded Wo.  Each core computes a
full-shape partial of the output projection; the host sums the 8 partials
(row-parallel "gather") and reshapes to (T, B, E).

Per-core dataflow (all on-chip except noted):
  1. qT/kT projections emitted *transposed* ([dims, tokens], dims on
     partitions) so the head_dim contraction of the attention matmuls needs no
     on-device transposes at all; v is emitted in natural layout ([tokens,
     dims]) to serve as the stationary operand of P@V.
  2. RoPE applied in transposed layout with precomputed cos/sin tables
     (attention scale folded into q's tables).
  3. Scores are computed transposed (sT[k, tq] = kT.T @ qT), bias added from a
     host-transposed attn_bias shard, exp on ACT without max-subtraction
     (logits are O(10), fp32 exp is exact enough), then oT = v.T @ p
     accumulates over k-blocks.  A ones-column appended to v yields the
     softmax denominator for free; the key-padding mask is folded into v rows
     so masked keys drop out of both numerator and denominator.
  4. Output projection emitted transposed ([e, tokens]) so bo is a
     per-partition ACT bias; DMA'd out as a [1024, 4096] partial.
"""

import os
import sys

for _p in ("/opt/trn_rl_repo", "/root/.axon_site/_ro/trn_rl_repo"):
    if os.path.isdir(_p) and _p not in sys.path:
        sys.path.insert(0, _p)

import numpy as np
from contextlib import ExitStack

import concourse.bass as bass
import concourse.bacc as bacc
import concourse.tile as tile
from concourse import mybir
from concourse.bass_utils import run_bass_kernel_spmd

F32 = mybir.dt.float32
U8 = mybir.dt.uint8

E = 1024
H = 16
D = 64
T = 1024
B = 4
P = 128
NCORES = 8
HPC = H // NCORES  # heads per core = 2
TB = T * B  # 4096 tokens, stored b-major on device
NT = TB // 512  # 8 token tiles of 512
SCALE = 1.0 / np.sqrt(np.float32(D))  # 0.125

# matmul dtype knob: "f32" (safe) or "f32r" (full-rate, reduced precision)
MM_DTYPE = os.environ.get("KERNEL_MM_DTYPE", "f32r")
DEBUG_TAPS = os.environ.get("KERNEL_DEBUG", "") == "1"


def _mm(ap):
    if MM_DTYPE == "f32r":
        return ap.bitcast(mybir.dt.float32r)
    return ap


def _build_bass():
    nc = bacc.Bacc("TRN2", target_bir_lowering=False, debug=False)

    # ---- per-core external inputs ----
    queryT = nc.dram_tensor("queryT", [E, TB], F32, kind="ExternalInput")
    biasT = nc.dram_tensor("biasT", [B * HPC, T, T], F32, kind="ExternalInput")
    wqT = nc.dram_tensor("wqT", [E, P], F32, kind="ExternalInput")
    wkT = nc.dram_tensor("wkT", [E, P], F32, kind="ExternalInput")
    wvT = nc.dram_tensor("wvT", [E, P], F32, kind="ExternalInput")
    woT = nc.dram_tensor("woT", [P, E], F32, kind="ExternalInput")
    bq_in = nc.dram_tensor("bq", [P, 1], F32, kind="ExternalInput")
    bv_in = nc.dram_tensor("bv", [1, P], F32, kind="ExternalInput")
    bo_in = nc.dram_tensor("bo", [P, 8], F32, kind="ExternalInput")
    mask_in = nc.dram_tensor("masku8", [B, T], U8, kind="ExternalInput")
    cos_q = nc.dram_tensor("cos_q", [P, T], F32, kind="ExternalInput")
    sin_q = nc.dram_tensor("sin_q", [P, T], F32, kind="ExternalInput")
    cos_k = nc.dram_tensor("cos_k", [P, T], F32, kind="ExternalInput")
    sin_k = nc.dram_tensor("sin_k", [P, T], F32, kind="ExternalInput")
    outT = nc.dram_tensor("outT", [E, TB], F32, kind="ExternalOutput")
    dbg = {}
    if DEBUG_TAPS:
        for name, shape in (
            ("dbg_keep", [P, TB // P]),
            ("dbg_v", [P, 2 * (D + 2)]),
            ("dbg_qT", [P, 512]),
            ("dbg_kT", [P, 512]),
            ("dbg_l", [B * HPC, T]),
            ("dbg_rcp", [B * HPC, T]),
            ("dbg_p", [P, T]),
            ("dbg_s", [P, T]),
        ):
            dbg[name] = nc.dram_tensor(name, shape, F32, kind="ExternalOutput")

    Exp = mybir.ActivationFunctionType.Exp
    Identity = mybir.ActivationFunctionType.Identity

    with tile.TileContext(nc) as tc, ExitStack() as ctx:
        # ---------------- persistent tiles ----------------
        persist = ctx.enter_context(tc.tile_pool(name="persist", bufs=1))
        qT_sb = persist.tile([P, TB], F32)  # roped, scaled q^T (2 heads)
        kT_sb = persist.tile([P, TB], F32)  # roped k^T
        # v in natural layout with a ones column per head:
        # [tok128, tile, 66*2] : cols 0:64 = head0 dims, 64 = ones,
        #                        cols 66:130 = head1 dims, 130 = ones
        v_sb = persist.tile([P, TB // P, 2 * (D + 2)], F32)
        oT_sb = persist.tile([P, TB], F32)  # normalized attention out^T
        wo_sb = persist.tile([P, 8, P], F32)
        bo_sb = persist.tile([P, 8], F32)

        nc.sync.dma_start(out=wo_sb[:], in_=woT.ap().rearrange("p (c m) -> p c m", m=P))
        nc.sync.dma_start(out=bo_sb[:], in_=bo_in[:])

        with tc.tile_pool(name="proj_consts", bufs=1) as consts:
            wq_sb = consts.tile([P, 8, P], F32)
            wk_sb = consts.tile([P, 8, P], F32)
            wv_sb = consts.tile([P, 8, P], F32)
            # weight DRAM layout [E, 128] -> SBUF [128, kchunk, 128]
            for w_sb, w_dram in ((wq_sb, wqT), (wk_sb, wkT), (wv_sb, wvT)):
                nc.sync.dma_start(
                    out=w_sb[:], in_=w_dram.ap().rearrange("(c p) m -> p c m", p=P)
                )
            bq_sb = consts.tile([P, 1], F32)
            nc.sync.dma_start(out=bq_sb[:], in_=bq_in[:])
            bv_sb = consts.tile([P, P], F32)  # bv broadcast along partitions
            nc.sync.dma_start(
                out=bv_sb[:],
                in_=bass.AP(tensor=bv_in, offset=0, ap=[[0, P], [1, P]]),
            )
            cq_sb = consts.tile([P, T], F32)
            sq_sb = consts.tile([P, T], F32)
            ck_sb = consts.tile([P, T], F32)
            sk_sb = consts.tile([P, T], F32)
            for t_sb, t_dram in (
                (cq_sb, cos_q),
                (sq_sb, sin_q),
                (ck_sb, cos_k),
                (sk_sb, sin_k),
            ):
                nc.sync.dma_start(out=t_sb[:], in_=t_dram[:])
            # key padding mask -> keep factor, transposed: keepT[p, ti] =
            # 1 - mask[b, tc*128 + p] with ti = b*8 + tc (b-major token tiles)
            masku8_sb = consts.tile([P, TB // P], U8)
            nc.sync.dma_start(
                out=masku8_sb[:],
                in_=bass.AP(tensor=mask_in, offset=0, ap=[[1, P], [T, B], [P, 8]]),
            )
            keepT = consts.tile([P, TB // P], F32)
            nc.vector.tensor_scalar(
                out=keepT[:],
                in0=masku8_sb[:],
                scalar1=-1.0,
                scalar2=1.0,
                op0=mybir.AluOpType.mult,
                op1=mybir.AluOpType.add,
            )
            if DEBUG_TAPS:
                nc.sync.dma_start(out=dbg["dbg_keep"][:], in_=keepT[:])

            # ---------------- phase 1: projections + rope ----------------
            with (
                tc.tile_pool(name="qry", bufs=2) as qry_pool,
                tc.tile_pool(name="praw", bufs=3) as praw_pool,
                tc.tile_pool(name="ptmp", bufs=3) as ptmp_pool,
                tc.tile_pool(name="pj_psum", bufs=2, space="PSUM") as pj_psum,
                tc.tile_pool(name="pv_psum", bufs=2, space="PSUM") as pv_psum,
            ):
                for nt in range(NT):
                    sl = slice(nt * 512, (nt + 1) * 512)
                    qry = qry_pool.tile([P, 8, 512], F32)
                    nc.sync.dma_start(
                        out=qry[:],
                        in_=bass.AP(
                            tensor=queryT,
                            offset=nt * 512,
                            ap=[[TB, P], [P * TB, 8], [1, 512]],
                        ),
                    )
                    for which, w_sb in (("q", wq_sb), ("k", wk_sb)):
                        ps = pj_psum.tile([P, 512], F32, tag="pj")
                        for k in range(8):
                            nc.tensor.matmul(
                                ps[:],
                                lhsT=_mm(w_sb[:, k, :]),
                                rhs=_mm(qry[:, k, :]),
                                start=(k == 0),
                                stop=(k == 7),
                            )
                        raw = praw_pool.tile([P, 512], F32)
                        if which == "q":
                            # copyback with projection bias (ACT, per-partition)
                            nc.scalar.activation(
                                raw[:], ps[:], Identity, bias=bq_sb[:], scale=1.0
                            )
                            csb, ssb, dst = cq_sb, sq_sb, qT_sb
                        else:
                            nc.scalar.copy(raw[:], ps[:])
                            csb, ssb, dst = ck_sb, sk_sb, kT_sb
                        # rope: dst = raw * cos + rot(raw) * sin
                        # sin tables are pre-signed (rows 0:32 of each head
                        # carry -sin); q tables are pre-scaled by 1/sqrt(D).
                        # token index within the 512-slice maps to absolute
                        # t = (token % 1024); b-major slices keep t contiguous
                        # in blocks of 1024, and 512 | 1024 so the table
                        # column range is (nt*512) % 1024 .. +512.
                        tsl = slice((nt * 512) % T, (nt * 512) % T + 512)
                        # sin tables are indexed by *input* row (rows 0:32 of
                        # each head carry +sin, rows 32:64 carry -sin) so both
                        # SBUF inputs share a base partition; only the output
                        # is partition-shifted (walrus requires SB-SB inputs
                        # to share a start partition).
                        tmp = ptmp_pool.tile([P, 512], F32)
                        for h0 in (0, 64):
                            nc.gpsimd.tensor_mul(
                                tmp[h0 : h0 + 32, :],
                                raw[h0 + 32 : h0 + 64, :],
                                ssb[h0 + 32 : h0 + 64, tsl],
                            )
                            nc.gpsimd.tensor_mul(
                                tmp[h0 + 32 : h0 + 64, :],
                                raw[h0 : h0 + 32, :],
                                ssb[h0 : h0 + 32, tsl],
                            )
                        nc.vector.tensor_mul(dst[:, sl], raw[:], csb[:, tsl])
                        nc.vector.tensor_add(dst[:, sl], dst[:, sl], tmp[:])
                    # v projection: natural layout, 4 token sub-tiles of 128
                    for j in range(4):
                        ti = nt * 4 + j
                        psv = pv_psum.tile([P, P], F32, tag="pv")
                        for k in range(8):
                            nc.tensor.matmul(
                                psv[:],
                                lhsT=_mm(qry[:, k, j * P : (j + 1) * P]),
                                rhs=_mm(wv_sb[:, k, :]),
                                start=(k == 0),
                                stop=(k == 7),
                            )
                        nc.vector.tensor_add(
                            v_sb[:, ti, 0:D], psv[:, 0:D], bv_sb[:, 0:D]
                        )
                        nc.vector.tensor_add(
                            v_sb[:, ti, D + 2 : 2 * D + 2],
                            psv[:, D : 2 * D],
                            bv_sb[:, D : 2 * D],
                        )
                        nc.vector.memset(v_sb[:, ti, D : 2 * (D + 2) : D + 2], 1.0)
                        nc.vector.memset(
                            v_sb[:, ti, D + 1 : 2 * (D + 2) : D + 2], 0.0
                        )
                        # fold key-padding mask into v rows (and ones column),
                        # so masked keys vanish from numerator + denominator
                        nc.vector.tensor_scalar_mul(
                            v_sb[:, ti, :], v_sb[:, ti, :], keepT[:, ti : ti + 1]
                        )
                        if DEBUG_TAPS and ti == 0:
                            nc.sync.dma_start(
                                out=dbg["dbg_v"][:], in_=v_sb[:, 0, :]
                            )
                if DEBUG_TAPS:
                    nc.sync.dma_start(out=dbg["dbg_qT"][:], in_=qT_sb[:, 0:512])
                    nc.sync.dma_start(out=dbg["dbg_kT"][:], in_=kT_sb[:, 0:512])

        # ---------------- phase 2: attention ----------------
        with (
            tc.tile_pool(name="sbias", bufs=3) as bias_pool,
            tc.tile_pool(name="ssb", bufs=2) as s_pool,
            tc.tile_pool(name="pp", bufs=2) as p_pool,
            tc.tile_pool(name="rcp", bufs=2) as rcp_pool,
            tc.tile_pool(name="s_psum", bufs=2, space="PSUM") as s_psum,
            tc.tile_pool(name="o_psum", bufs=2, space="PSUM") as o_psum,
        ):
            for b in range(B):
                for h in range(HPC):
                    bh = b * HPC + h
                    hsl = slice(h * D, (h + 1) * D)
                    bsl = slice(b * T, (b + 1) * T)
                    o_ps = o_psum.tile([P, T], F32, tag="ops")
                    for kb in range(8):
                        bias_t = bias_pool.tile([P, T], F32)
                        nc.sync.dma_start(
                            out=bias_t[:], in_=biasT[bh, kb * P : (kb + 1) * P, :]
                        )
                        s_ps = s_psum.tile([P, T], F32, tag="sps")
                        for half in range(2):
                            nc.tensor.matmul(
                                s_ps[:, half * 512 : (half + 1) * 512],
                                lhsT=_mm(kT_sb[hsl, b * T + kb * P : b * T + (kb + 1) * P]),
                                rhs=_mm(
                                    qT_sb[hsl, b * T + half * 512 : b * T + (half + 1) * 512]
                                ),
                                start=True,
                                stop=True,
                            )
                        s_sb = s_pool.tile([P, T], F32)
                        nc.vector.tensor_add(s_sb[:], s_ps[:], bias_t[:])
                        p_t = p_pool.tile([P, T], F32)
                        nc.scalar.activation(p_t[:], s_sb[:], Exp)
                        if DEBUG_TAPS and bh == 0 and kb == 0:
                            nc.sync.dma_start(out=dbg["dbg_s"][:], in_=s_sb[:])
                            nc.sync.dma_start(out=dbg["dbg_p"][:], in_=p_t[:])
                        for half in range(2):
                            nc.tensor.matmul(
                                o_ps[0 : D + 1, half * 512 : (half + 1) * 512],
                                lhsT=_mm(
                                    v_sb[:, b * 8 + kb, h * (D + 2) : h * (D + 2) + D + 1]
                                ),
                                rhs=_mm(p_t[:, half * 512 : (half + 1) * 512]),
                                start=(kb == 0),
                                stop=(kb == 7),
                            )
                    # normalize: oT = o_unnorm * (1/l) broadcast over dims
                    # copy l to SBUF partition 0 first: reciprocal_approx_fast
                    # miscomputes on a partition-shifted PSUM input (HW bug,
                    # verified by probe)
                    l_sb = rcp_pool.tile([1, T], F32, tag="lsb")
                    nc.vector.tensor_copy(l_sb[:], o_ps[D : D + 1, :])
                    rcp_row = rcp_pool.tile([1, T], F32, tag="rrow")
                    nc.vector.reciprocal_approx_fast(rcp_row[:], l_sb[:])
                    rcp_b = rcp_pool.tile([D, T], F32, tag="rbc")
                    nc.gpsimd.partition_broadcast(rcp_b[:], rcp_row[:])
                    if DEBUG_TAPS:
                        nc.sync.dma_start(out=dbg["dbg_l"][bh : bh + 1, :], in_=l_sb[:])
                        nc.sync.dma_start(
                            out=dbg["dbg_rcp"][bh : bh + 1, :], in_=rcp_row[:]
                        )
                    nc.vector.tensor_mul(oT_sb[hsl, bsl], o_ps[0:D, :], rcp_b[:])

        # ---------------- phase 3: output projection ----------------
        with (
            tc.tile_pool(name="orow", bufs=2) as orow_pool,
            tc.tile_pool(name="f_psum", bufs=2, space="PSUM") as f_psum,
        ):
            for et in range(8):
                orow = orow_pool.tile([P, TB], F32)
                for ntt in range(NT):
                    ps = f_psum.tile([P, 512], F32, tag="fps")
                    nc.tensor.matmul(
                        ps[:],
                        lhsT=_mm(wo_sb[:, et, :]),
                        rhs=_mm(oT_sb[:, ntt * 512 : (ntt + 1) * 512]),
                        start=True,
                        stop=True,
                    )
                    nc.scalar.activation(
                        orow[:, ntt * 512 : (ntt + 1) * 512],
                        ps[:],
                        Identity,
                        bias=bo_sb[:, et : et + 1],
                        scale=1.0,
                    )
                nc.sync.dma_start(out=outT[et * P : (et + 1) * P, :], in_=orow[:])

    nc.compile()
    return nc


_NC_CACHE = None


def _get_nc():
    global _NC_CACHE
    if _NC_CACHE is None:
        _NC_CACHE = _build_bass()
    return _NC_CACHE


def _rope_tables():
    """cos/sin tables in [dim(128, 2 heads stacked), t] layout.

    sin is indexed by *input* row: rows 0:32 of each 64-row head block carry
    +sin (they feed output rows 32:64), rows 32:64 carry -sin (feeding output
    rows 0:32 with rotate_half's negation).  q tables are pre-scaled by the
    attention scale.
    """
    d = np.arange(0, D, 2, dtype=np.float32) / np.float32(D)
    inv_freq = (np.float32(1.0) / np.power(np.float32(10000.0), d)).astype(np.float32)
    t = np.arange(T, dtype=np.float32)
    freqs = t[None, :] * inv_freq[:, None]  # [32, T]
    cos_h = np.cos(np.concatenate([freqs, freqs], axis=0)).astype(np.float32)  # [64,T]
    sin_half = np.sin(freqs).astype(np.float32)
    sin_signed = np.concatenate([sin_half, -sin_half], axis=0)  # [64, T]
    cos = np.vstack([cos_h, cos_h])  # [128, T] (2 heads)
    sin = np.vstack([sin_signed, sin_signed])
    s = np.float32(SCALE)
    return (
        np.ascontiguousarray(cos * s),
        np.ascontiguousarray(sin * s),
        np.ascontiguousarray(cos),
        np.ascontiguousarray(sin),
    )


def _make_in_maps(query, attn_bias, key_padding_mask, Wq, bq, Wk, Wv, bv, Wo, bo):
    query = np.asarray(query, dtype=np.float32)
    attn_bias = np.asarray(attn_bias, dtype=np.float32)
    key_padding_mask = np.asarray(key_padding_mask)
    Wq = np.asarray(Wq, dtype=np.float32)
    Wk = np.asarray(Wk, dtype=np.float32)
    Wv = np.asarray(Wv, dtype=np.float32)
    Wo = np.asarray(Wo, dtype=np.float32)
    bq = np.asarray(bq, dtype=np.float32)
    bv = np.asarray(bv, dtype=np.float32)
    bo = np.asarray(bo, dtype=np.float32)

    # shared across cores
    queryT = np.ascontiguousarray(query.transpose(2, 1, 0).reshape(E, TB))
    masku8 = np.ascontiguousarray(key_padding_mask.astype(np.uint8))
    cos_q, sin_q, cos_k, sin_k = _rope_tables()
    bo_zero = np.zeros((P, 8), dtype=np.float32)
    bo_col = np.ascontiguousarray(bo.reshape(8, P).T)  # [p, echunk]

    in_maps = []
    for c in range(NCORES):
        rsl = slice(c * P, (c + 1) * P)
        in_maps.append(
            {
                "queryT": queryT,
                "biasT": np.ascontiguousarray(
                    attn_bias[:, c * HPC : (c + 1) * HPC].transpose(0, 1, 3, 2)
                ).reshape(B * HPC, T, T),
                "wqT": np.ascontiguousarray(Wq[rsl, :].T),
                "wkT": np.ascontiguousarray(Wk[rsl, :].T),
                "wvT": np.ascontiguousarray(Wv[rsl, :].T),
                "woT": np.ascontiguousarray(Wo[:, rsl].T),
                "bq": np.ascontiguousarray(bq[rsl].reshape(P, 1)),
                "bv": np.ascontiguousarray(bv[rsl].reshape(1, P)),
                "bo": bo_col if c == 0 else bo_zero,
                "masku8": masku8,
                "cos_q": cos_q,
                "sin_q": sin_q,
                "cos_k": cos_k,
                "sin_k": sin_k,
            }
        )
    return in_maps


def _run(inputs, trace=False, **kwargs):
    nc = _get_nc()
    in_maps = _make_in_maps(**inputs)
    res = run_bass_kernel_spmd(
        nc, in_maps, core_ids=list(range(NCORES)), trace=trace, **kwargs
    )
    acc = np.zeros((E, TB), dtype=np.float32)
    for r in res.results:
        acc += r["outT"]
    out = np.ascontiguousarray(acc.reshape(E, B, T).transpose(2, 1, 0))
    return out, res


def kernel(**inputs) -> np.ndarray:
    out, _ = _run(inputs, trace=False)
    return out


# revision 14
# speedup vs baseline: 1.4484x; 1.4484x over previous
"""Trainium2 Bass kernel for nn_MemEffAttn (T=1024, B=4, E=1024, H=16, D=64).

Sharding (8 cores): head-parallel attention (2 heads x 4 batches per core),
Megatron-style column-sharded Wq/Wk/Wv, row-sharded Wo.  Each core computes a
full-shape partial of the output projection; the host sums the 8 partials
(row-parallel "gather") and reshapes to (T, B, E).

Per-core dataflow (all on-chip except noted):
  1. qT/kT projections emitted *transposed* ([dims, tokens], dims on
     partitions) so the head_dim contraction of the attention matmuls needs no
     on-device transposes at all; v is emitted in natural layout ([tokens,
     dims]) to serve as the stationary operand of P@V.
  2. RoPE applied in transposed layout with precomputed cos/sin tables
     (attention scale folded into q's tables).
  3. Scores are computed transposed (sT[k, tq] = kT.T @ qT), bias added from a
     host-transposed attn_bias shard, exp on ACT without max-subtraction
     (logits are O(10), fp32 exp is exact enough), then oT = v.T @ p
     accumulates over k-blocks.  A ones-column appended to v yields the
     softmax denominator for free; the key-padding mask is folded into v rows
     so masked keys drop out of both numerator and denominator.
  4. Output projection emitted transposed ([e, tokens]) so bo is a
     per-partition ACT bias; DMA'd out as a [1024, 4096] partial.
"""

import os
import sys

for _p in ("/opt/trn_rl_repo", "/root/.axon_site/_ro/trn_rl_repo"):
    if os.path.isdir(_p) and _p not in sys.path:
        sys.path.insert(0, _p)

import numpy as np
from contextlib import ExitStack

import concourse.bass as bass
import concourse.bacc as bacc
import concourse.tile as tile
from concourse import mybir
from concourse.bass_utils import run_bass_kernel_spmd

F32 = mybir.dt.float32
U8 = mybir.dt.uint8

E = 1024
H = 16
D = 64
T = 1024
B = 4
P = 128
NCORES = 8
HPC = H // NCORES  # heads per core = 2
TB = T * B  # 4096 tokens, stored b-major on device
NT = TB // 512  # 8 token tiles of 512
SCALE = 1.0 / np.sqrt(np.float32(D))  # 0.125

# matmul dtype knob: "f32" (safe) or "f32r" (full-rate, reduced precision)
MM_DTYPE = os.environ.get("KERNEL_MM_DTYPE", "f32r")
DEBUG_TAPS = os.environ.get("KERNEL_DEBUG", "") == "1"


MMDT = mybir.dt.float32r if MM_DTYPE == "f32r" else F32


def _mm(ap):
    return ap


def _build_bass():
    nc = bacc.Bacc("TRN2", target_bir_lowering=False, debug=False)

    # ---- per-core external inputs ----
    queryT = nc.dram_tensor("queryT", [E, TB], F32, kind="ExternalInput")
    biasT = nc.dram_tensor("biasT", [B * HPC, T, T], F32, kind="ExternalInput")
    wqT = nc.dram_tensor("wqT", [E, P], F32, kind="ExternalInput")
    wkT = nc.dram_tensor("wkT", [E, P], F32, kind="ExternalInput")
    wvT = nc.dram_tensor("wvT", [E, P], F32, kind="ExternalInput")
    woT = nc.dram_tensor("woT", [P, E], F32, kind="ExternalInput")
    bq_in = nc.dram_tensor("bq", [P, 1], F32, kind="ExternalInput")
    bv_in = nc.dram_tensor("bv", [1, P], F32, kind="ExternalInput")
    bo_in = nc.dram_tensor("bo", [P, 8], F32, kind="ExternalInput")
    mask_in = nc.dram_tensor("masku8", [B, T], U8, kind="ExternalInput")
    cos_q = nc.dram_tensor("cos_q", [P, T], F32, kind="ExternalInput")
    sin_q = nc.dram_tensor("sin_q", [P, T], F32, kind="ExternalInput")
    cos_k = nc.dram_tensor("cos_k", [P, T], F32, kind="ExternalInput")
    sin_k = nc.dram_tensor("sin_k", [P, T], F32, kind="ExternalInput")
    outT = nc.dram_tensor("outT", [E, TB], F32, kind="ExternalOutput")
    dbg = {}
    if DEBUG_TAPS:
        for name, shape in (
            ("dbg_keep", [P, TB // P]),
            ("dbg_v", [P, 2 * (D + 2)]),
            ("dbg_qT", [P, 512]),
            ("dbg_kT", [P, 512]),
            ("dbg_l", [B * HPC, T]),
            ("dbg_rcp", [B * HPC, T]),
            ("dbg_p", [P, T]),
            ("dbg_s", [P, T]),
        ):
            dbg[name] = nc.dram_tensor(name, shape, F32, kind="ExternalOutput")

    Exp = mybir.ActivationFunctionType.Exp
    Identity = mybir.ActivationFunctionType.Identity

    with tile.TileContext(nc) as tc, ExitStack() as ctx:
        # ---------------- persistent tiles ----------------
        persist = ctx.enter_context(tc.tile_pool(name="persist", bufs=1))
        qT_sb = persist.tile([P, TB], MMDT)  # roped, scaled q^T (2 heads)
        kT_sb = persist.tile([P, TB], MMDT)  # roped k^T
        # v in natural layout with a ones column per head:
        # [tok128, tile, 66*2] : cols 0:64 = head0 dims, 64 = ones,
        #                        cols 66:130 = head1 dims, 130 = ones
        v_sb = persist.tile([P, TB // P, 2 * (D + 2)], MMDT)
        oT_sb = persist.tile([P, TB], MMDT)  # normalized attention out^T
        wo_sb = persist.tile([P, 8, P], MMDT)
        bo_sb = persist.tile([P, 8], F32)

        _wdma = nc.gpsimd if MM_DTYPE == "f32r" else nc.sync
        _wdma.dma_start(out=wo_sb[:], in_=woT.ap().rearrange("p (c m) -> p c m", m=P))
        nc.sync.dma_start(out=bo_sb[:], in_=bo_in[:])

        with tc.tile_pool(name="proj_consts", bufs=1) as consts:
            wq_sb = consts.tile([P, 8, P], MMDT)
            wk_sb = consts.tile([P, 8, P], MMDT)
            wv_sb = consts.tile([P, 8, P], MMDT)
            # weight DRAM layout [E, 128] -> SBUF [128, kchunk, 128]
            for w_sb, w_dram in ((wq_sb, wqT), (wk_sb, wkT), (wv_sb, wvT)):
                _wdma.dma_start(
                    out=w_sb[:], in_=w_dram.ap().rearrange("(c p) m -> p c m", p=P)
                )
            bq_sb = consts.tile([P, 1], F32)
            nc.sync.dma_start(out=bq_sb[:], in_=bq_in[:])
            bv_sb = consts.tile([P, P], F32)  # bv broadcast along partitions
            nc.sync.dma_start(
                out=bv_sb[:],
                in_=bass.AP(tensor=bv_in, offset=0, ap=[[0, P], [1, P]]),
            )
            cq_sb = consts.tile([P, T], F32)
            sq_sb = consts.tile([P, T], F32)
            ck_sb = consts.tile([P, T], F32)
            sk_sb = consts.tile([P, T], F32)
            for t_sb, t_dram in (
                (cq_sb, cos_q),
                (sq_sb, sin_q),
                (ck_sb, cos_k),
                (sk_sb, sin_k),
            ):
                nc.sync.dma_start(out=t_sb[:], in_=t_dram[:])
            # key padding mask -> keep factor, transposed: keepT[p, ti] =
            # 1 - mask[b, tc*128 + p] with ti = b*8 + tc (b-major token tiles)
            masku8_sb = consts.tile([P, TB // P], U8)
            nc.sync.dma_start(
                out=masku8_sb[:],
                in_=bass.AP(tensor=mask_in, offset=0, ap=[[1, P], [T, B], [P, 8]]),
            )
            keepT = consts.tile([P, TB // P], F32)
            nc.vector.tensor_scalar(
                out=keepT[:],
                in0=masku8_sb[:],
                scalar1=-1.0,
                scalar2=1.0,
                op0=mybir.AluOpType.mult,
                op1=mybir.AluOpType.add,
            )
            if DEBUG_TAPS:
                nc.sync.dma_start(out=dbg["dbg_keep"][:], in_=keepT[:])

            # ---------------- phase 1: projections + rope ----------------
            with (
                tc.tile_pool(name="qry", bufs=2) as qry_pool,
                tc.tile_pool(name="praw", bufs=3) as praw_pool,
                tc.tile_pool(name="ptmp", bufs=3) as ptmp_pool,
                tc.tile_pool(name="pj_psum", bufs=2, space="PSUM") as pj_psum,
                tc.tile_pool(name="pv_psum", bufs=2, space="PSUM") as pv_psum,
            ):
                for nt in range(NT):
                    sl = slice(nt * 512, (nt + 1) * 512)
                    qry = qry_pool.tile([P, 8, 512], MMDT)
                    _wdma.dma_start(
                        out=qry[:],
                        in_=bass.AP(
                            tensor=queryT,
                            offset=nt * 512,
                            ap=[[TB, P], [P * TB, 8], [1, 512]],
                        ),
                    )
                    for which, w_sb in (("q", wq_sb), ("k", wk_sb)):
                        ps = pj_psum.tile([P, 512], F32, tag="pj")
                        for k in range(8):
                            nc.tensor.matmul(
                                ps[:],
                                lhsT=_mm(w_sb[:, k, :]),
                                rhs=_mm(qry[:, k, :]),
                                start=(k == 0),
                                stop=(k == 7),
                            )
                        raw = praw_pool.tile([P, 512], F32)
                        if which == "q":
                            # copyback with projection bias (ACT, per-partition)
                            nc.scalar.activation(
                                raw[:], ps[:], Identity, bias=bq_sb[:], scale=1.0
                            )
                            csb, ssb, dst = cq_sb, sq_sb, qT_sb
                        else:
                            nc.scalar.copy(raw[:], ps[:])
                            csb, ssb, dst = ck_sb, sk_sb, kT_sb
                        # rope: dst = raw * cos + rot(raw) * sin
                        # sin tables are pre-signed (rows 0:32 of each head
                        # carry -sin); q tables are pre-scaled by 1/sqrt(D).
                        # token index within the 512-slice maps to absolute
                        # t = (token % 1024); b-major slices keep t contiguous
                        # in blocks of 1024, and 512 | 1024 so the table
                        # column range is (nt*512) % 1024 .. +512.
                        tsl = slice((nt * 512) % T, (nt * 512) % T + 512)
                        # sin tables are indexed by *input* row (rows 0:32 of
                        # each head carry +sin, rows 32:64 carry -sin) so both
                        # SBUF inputs share a base partition; only the output
                        # is partition-shifted (walrus requires SB-SB inputs
                        # to share a start partition).
                        tmp = ptmp_pool.tile([P, 512], F32)
                        for h0 in (0, 64):
                            nc.gpsimd.tensor_mul(
                                tmp[h0 : h0 + 32, :],
                                raw[h0 + 32 : h0 + 64, :],
                                ssb[h0 + 32 : h0 + 64, tsl],
                            )
                            nc.gpsimd.tensor_mul(
                                tmp[h0 + 32 : h0 + 64, :],
                                raw[h0 : h0 + 32, :],
                                ssb[h0 : h0 + 32, tsl],
                            )
                        nc.vector.tensor_mul(dst[:, sl], raw[:], csb[:, tsl])
                        nc.vector.tensor_add(dst[:, sl], dst[:, sl], tmp[:])
                    # v projection: natural layout, 4 token sub-tiles of 128
                    for j in range(4):
                        ti = nt * 4 + j
                        psv = pv_psum.tile([P, P], F32, tag="pv")
                        for k in range(8):
                            nc.tensor.matmul(
                                psv[:],
                                lhsT=_mm(qry[:, k, j * P : (j + 1) * P]),
                                rhs=_mm(wv_sb[:, k, :]),
                                start=(k == 0),
                                stop=(k == 7),
                            )
                        nc.vector.tensor_add(
                            v_sb[:, ti, 0:D], psv[:, 0:D], bv_sb[:, 0:D]
                        )
                        nc.vector.tensor_add(
                            v_sb[:, ti, D + 2 : 2 * D + 2],
                            psv[:, D : 2 * D],
                            bv_sb[:, D : 2 * D],
                        )
                        nc.vector.memset(
                            v_sb[:, ti, D : 2 * (D + 2) : D + 2].bitcast(F32), 1.0
                        )
                        nc.vector.memset(
                            v_sb[:, ti, D + 1 : 2 * (D + 2) : D + 2].bitcast(F32), 0.0
                        )
                        # fold key-padding mask into v rows (and ones column),
                        # so masked keys vanish from numerator + denominator
                        nc.vector.tensor_scalar_mul(
                            v_sb[:, ti, :], v_sb[:, ti, :], keepT[:, ti : ti + 1]
                        )
                        if DEBUG_TAPS and ti == 0:
                            nc.sync.dma_start(
                                out=dbg["dbg_v"][:], in_=v_sb[:, 0, :].bitcast(F32)
                            )
                if DEBUG_TAPS:
                    nc.sync.dma_start(out=dbg["dbg_qT"][:], in_=qT_sb[:, 0:512].bitcast(F32))
                    nc.sync.dma_start(out=dbg["dbg_kT"][:], in_=kT_sb[:, 0:512].bitcast(F32))

        # ---------------- phase 2: attention ----------------
        with (
            tc.tile_pool(name="sbias", bufs=3) as bias_pool,
            tc.tile_pool(name="ssb", bufs=2) as s_pool,
            tc.tile_pool(name="pp", bufs=2) as p_pool,
            tc.tile_pool(name="rcp", bufs=2) as rcp_pool,
            tc.tile_pool(name="s_psum", bufs=2, space="PSUM") as s_psum,
            tc.tile_pool(name="o_psum", bufs=2, space="PSUM") as o_psum,
        ):
            for b in range(B):
                for h in range(HPC):
                    bh = b * HPC + h
                    hsl = slice(h * D, (h + 1) * D)
                    bsl = slice(b * T, (b + 1) * T)
                    o_ps = o_psum.tile([P, T], F32, tag="ops")
                    for kb in range(8):
                        bias_t = bias_pool.tile([P, T], F32)
                        nc.sync.dma_start(
                            out=bias_t[:], in_=biasT[bh, kb * P : (kb + 1) * P, :]
                        )
                        s_ps = s_psum.tile([P, T], F32, tag="sps")
                        for half in range(2):
                            nc.tensor.matmul(
                                s_ps[:, half * 512 : (half + 1) * 512],
                                lhsT=_mm(kT_sb[hsl, b * T + kb * P : b * T + (kb + 1) * P]),
                                rhs=_mm(
                                    qT_sb[hsl, b * T + half * 512 : b * T + (half + 1) * 512]
                                ),
                                start=True,
                                stop=True,
                            )
                        s_sb = s_pool.tile([P, T], F32)
                        nc.vector.tensor_add(s_sb[:], s_ps[:], bias_t[:])
                        p_t = p_pool.tile([P, T], MMDT)
                        nc.scalar.activation(p_t[:], s_sb[:], Exp)
                        if DEBUG_TAPS and bh == 0 and kb == 0:
                            nc.sync.dma_start(out=dbg["dbg_s"][:], in_=s_sb[:])
                            nc.sync.dma_start(out=dbg["dbg_p"][:], in_=p_t[:].bitcast(F32))
                        for half in range(2):
                            nc.tensor.matmul(
                                o_ps[0 : D + 1, half * 512 : (half + 1) * 512],
                                lhsT=_mm(
                                    v_sb[:, b * 8 + kb, h * (D + 2) : h * (D + 2) + D + 1]
                                ),
                                rhs=_mm(p_t[:, half * 512 : (half + 1) * 512]),
                                start=(kb == 0),
                                stop=(kb == 7),
                            )
                    # normalize: oT = o_unnorm * (1/l) broadcast over dims
                    # copy l to SBUF partition 0 first: reciprocal_approx_fast
                    # miscomputes on a partition-shifted PSUM input (HW bug,
                    # verified by probe)
                    l_sb = rcp_pool.tile([1, T], F32, tag="lsb")
                    nc.vector.tensor_copy(l_sb[:], o_ps[D : D + 1, :])
                    rcp_row = rcp_pool.tile([1, T], F32, tag="rrow")
                    nc.vector.reciprocal_approx_fast(rcp_row[:], l_sb[:])
                    rcp_b = rcp_pool.tile([D, T], F32, tag="rbc")
                    nc.gpsimd.partition_broadcast(rcp_b[:], rcp_row[:])
                    if DEBUG_TAPS:
                        nc.sync.dma_start(out=dbg["dbg_l"][bh : bh + 1, :], in_=l_sb[:])
                        nc.sync.dma_start(
                            out=dbg["dbg_rcp"][bh : bh + 1, :], in_=rcp_row[:]
                        )
                    nc.vector.tensor_mul(oT_sb[hsl, bsl], o_ps[0:D, :], rcp_b[:])

        # ---------------- phase 3: output projection ----------------
        with (
            tc.tile_pool(name="orow", bufs=2) as orow_pool,
            tc.tile_pool(name="f_psum", bufs=2, space="PSUM") as f_psum,
        ):
            for et in range(8):
                orow = orow_pool.tile([P, TB], F32)
                for ntt in range(NT):
                    ps = f_psum.tile([P, 512], F32, tag="fps")
                    nc.tensor.matmul(
                        ps[:],
                        lhsT=_mm(wo_sb[:, et, :]),
                        rhs=_mm(oT_sb[:, ntt * 512 : (ntt + 1) * 512]),
                        start=True,
                        stop=True,
                    )
                    nc.scalar.activation(
                        orow[:, ntt * 512 : (ntt + 1) * 512],
                        ps[:],
                        Identity,
                        bias=bo_sb[:, et : et + 1],
                        scale=1.0,
                    )
                nc.sync.dma_start(out=outT[et * P : (et + 1) * P, :], in_=orow[:])

    nc.compile()
    return nc


_NC_CACHE = None


def _get_nc():
    global _NC_CACHE
    if _NC_CACHE is None:
        _NC_CACHE = _build_bass()
    return _NC_CACHE


def _rope_tables():
    """cos/sin tables in [dim(128, 2 heads stacked), t] layout.

    sin is indexed by *input* row: rows 0:32 of each 64-row head block carry
    +sin (they feed output rows 32:64), rows 32:64 carry -sin (feeding output
    rows 0:32 with rotate_half's negation).  q tables are pre-scaled by the
    attention scale.
    """
    d = np.arange(0, D, 2, dtype=np.float32) / np.float32(D)
    inv_freq = (np.float32(1.0) / np.power(np.float32(10000.0), d)).astype(np.float32)
    t = np.arange(T, dtype=np.float32)
    freqs = t[None, :] * inv_freq[:, None]  # [32, T]
    cos_h = np.cos(np.concatenate([freqs, freqs], axis=0)).astype(np.float32)  # [64,T]
    sin_half = np.sin(freqs).astype(np.float32)
    sin_signed = np.concatenate([sin_half, -sin_half], axis=0)  # [64, T]
    cos = np.vstack([cos_h, cos_h])  # [128, T] (2 heads)
    sin = np.vstack([sin_signed, sin_signed])
    s = np.float32(SCALE)
    return (
        np.ascontiguousarray(cos * s),
        np.ascontiguousarray(sin * s),
        np.ascontiguousarray(cos),
        np.ascontiguousarray(sin),
    )


def _make_in_maps(query, attn_bias, key_padding_mask, Wq, bq, Wk, Wv, bv, Wo, bo):
    query = np.asarray(query, dtype=np.float32)
    attn_bias = np.asarray(attn_bias, dtype=np.float32)
    key_padding_mask = np.asarray(key_padding_mask)
    Wq = np.asarray(Wq, dtype=np.float32)
    Wk = np.asarray(Wk, dtype=np.float32)
    Wv = np.asarray(Wv, dtype=np.float32)
    Wo = np.asarray(Wo, dtype=np.float32)
    bq = np.asarray(bq, dtype=np.float32)
    bv = np.asarray(bv, dtype=np.float32)
    bo = np.asarray(bo, dtype=np.float32)

    # shared across cores
    queryT = np.ascontiguousarray(query.transpose(2, 1, 0).reshape(E, TB))
    masku8 = np.ascontiguousarray(key_padding_mask.astype(np.uint8))
    cos_q, sin_q, cos_k, sin_k = _rope_tables()
    bo_zero = np.zeros((P, 8), dtype=np.float32)
    bo_col = np.ascontiguousarray(bo.reshape(8, P).T)  # [p, echunk]

    in_maps = []
    for c in range(NCORES):
        rsl = slice(c * P, (c + 1) * P)
        in_maps.append(
            {
                "queryT": queryT,
                "biasT": np.ascontiguousarray(
                    attn_bias[:, c * HPC : (c + 1) * HPC].transpose(0, 1, 3, 2)
                ).reshape(B * HPC, T, T),
                "wqT": np.ascontiguousarray(Wq[rsl, :].T),
                "wkT": np.ascontiguousarray(Wk[rsl, :].T),
                "wvT": np.ascontiguousarray(Wv[rsl, :].T),
                "woT": np.ascontiguousarray(Wo[:, rsl].T),
                "bq": np.ascontiguousarray(bq[rsl].reshape(P, 1)),
                "bv": np.ascontiguousarray(bv[rsl].reshape(1, P)),
                "bo": bo_col if c == 0 else bo_zero,
                "masku8": masku8,
                "cos_q": cos_q,
                "sin_q": sin_q,
                "cos_k": cos_k,
                "sin_k": sin_k,
            }
        )
    return in_maps


def _run(inputs, trace=False, **kwargs):
    nc = _get_nc()
    in_maps = _make_in_maps(**inputs)
    res = run_bass_kernel_spmd(
        nc, in_maps, core_ids=list(range(NCORES)), trace=trace, **kwargs
    )
    acc = np.zeros((E, TB), dtype=np.float32)
    for r in res.results:
        acc += r["outT"]
    out = np.ascontiguousarray(acc.reshape(E, B, T).transpose(2, 1, 0))
    return out, res


def kernel(**inputs) -> np.ndarray:
    out, _ = _run(inputs, trace=False)
    return out
